# revision 23
# baseline (speedup 1.0000x reference)
"""Trainium2 Bass kernel for the MFA/MPPCA mixture log-likelihood problem.

Math: out[n,k] = PI[k] + logprob[n,k] with Sigma_k = A_k A_k^T + diag(D_k^2),
computed via Woodbury.  Everything involving only the small parameters
(MU, A, D, PI) is folded on the host into:

    out[n,k] = CONST[k] + x[n]·H[:,k] + (x[n]^2)·G[:,k] + sum_l (x[n]·Csc[:,k,l])^2

where (with iD = D^-2, B = iD*A, L = I + A^T B, iL = inv(L), R = chol(iL),
C0 = B R, e = R^T B^T MU):
    G   = -0.5 * iD^T                       (d, K)
    H   = (iD*MU)^T - C0 e                  (d, K)
    Csc = sqrt(0.5) * C0                    (d, K*l)
    CONST = PI - 0.5*(d log 2pi + logdet Sigma + MU^T iD MU) + 0.5 |e|^2

Device kernel (data-parallel over N on 8 cores; x and x^2 pre-transposed,
pre-tiled, and INTERLEAVED per 128-sample tile into one xz tensor on host):
  PE:     x·[H|Csc] as fp8e4 DoubleRowSwInterleave matmuls (256-deep
          contraction; stationary x tiles pre-interleaved/column-reversed on
          the host).  x^2·G as two fp8e4 SwInterleave matmuls accumulated
          into the psum H block.  One 2-bank psum tile per sample tile:
          [H+G 64 | Csc_a 320 | pad | Csc_b 320 @ +576].
          Steady state measures ~713ns/tile = the DR practical roofline
          (1536 logical cols x 1.13 / 2.4GHz); LDWEIGHTS fully hidden by the
          64-deep reorder window.
  Scalar: ONE activation squares all 640 factor projections per tile
          (dual 320-block psum AP at stride 512) -> sq tile (fp16).
  Vector: scalar_tensor_tensor folds psum H+G + CONST into r1 col 5
          (evacuating psum early so PE never stalls on psum bufs), then a
          single reduce over r1 [128,64,6] emits the fp16 output tile.
  GpSimd: pairwise pre-reduction of the 10 squares -> r1 cols 0:5.
Output is fp16, partition-major; host untransposes and casts to fp32.

Head/tail engineering (final): total exec ~= 7.2us framework preamble +
~4us DMA-bound head + 91.3us steady (128 tiles x 713ns, PE-bound at the
fp8-DR practical roofline: 1536 logical cols x ~1.11 / 2.4GHz, LDWEIGHTS
hidden by the PE's 64-deep reorder window) + ~3.4us post-chain drain +
~3.5us epilogue.  The head is bandwidth-bound (~270GB/s effective across
the two HWDGE queues; per-queue descriptor-limited: ~95GB/s at 1KB
per-partition runs, ~150 at 2KB, ~216 at 8KB; concurrent DMAs on a queue
round-robin their packets, spreading completions).  Layout: wall+G+CONST
tables partition-contiguous; wall halves fetched first on the scalar
queue; superblock-0 xz fills as 2-tile-pair DMAs on the sync queue;
superblock-1 on the scalar queue with its second half dependency-gated
behind tile-01's data (tiny gpsimd copy into its dst, overwritten by the
DMA) so it cannot steal early bandwidth; 33->31 warmup zero-matmuls
bridge the PE from the preamble to the first real matmul so HAM reaches
8/8 with no idle gap; steady superblocks are single 8KB-run DMAs on the
sync queue (scalar queue must stay DMA-free in steady state -- a waiting
DMA at that FIFO's head blocks the ACT squares).

Notes from HW tuning (trn2): plain DoubleRow loses (its 256-col LDWEIGHTS
does not background-load; LDW-bound), SwInterleave wins; two PSUM APs in
one DVE tensor_tensor is rejected by the BIR verifier; GpSimd has no PSUM
port; ACT runs 1 el/cycle/lane @1.2GHz; HWDGE queues exist on sync+scalar
only; a queue's first packet flows ~1.5-1.9us after its first issue.
"""
import math
import numpy as np
import ml_dtypes

N_TOTAL, K, D_FEAT, L_FAC = 131072, 64, 512, 10
N_CORES = 8
N_PER_CORE = N_TOTAL // N_CORES  # 16384

WALL_COLS = K + K * L_FAC  # 704 = [H (0:64) | Csc (64:704)]
WG_COLS = WALL_COLS + K    # 768 = [H | Csc | G (704:768)]
NGA = 32                   # factor groups in psum_a -> psum_a = 64 + 320 = 384
NGB = K - NGA              # factor groups in psum_b -> 320


def host_prep(MU, A, D, PI):
    """Fold small-parameter math into matmul weights (float64 internally)."""
    MU64, A64, D64, PI64 = [np.asarray(v, np.float64) for v in (MU, A, D, PI)]
    Kc, d, l = A64.shape
    iD = D64 ** -2.0
    B = iD[..., None] * A64
    L = np.eye(l)[None] + np.einsum('kdl,kdm->klm', A64, B)
    sign, logdet_L = np.linalg.slogdet(L)
    log_det_Sigma = logdet_L - np.sum(np.log(iD), axis=1)
    iL = np.linalg.inv(L)
    R = np.linalg.cholesky(iL)                  # R @ R.T = iL
    C0 = np.einsum('kdl,klm->kdm', B, R)        # (K, d, l)
    bmu = np.einsum('kdl,kd->kl', B, MU64)
    e = np.einsum('klm,kl->km', R, bmu)         # (K, l)
    c1 = np.sum(iD * MU64 * MU64, axis=1)

    CONST = PI64 - 0.5 * (d * math.log(2.0 * math.pi) + log_det_Sigma + c1) \
        + 0.5 * np.sum(e * e, axis=1)
    G = (-0.5 * iD).T
    H = (iD * MU64 - np.einsum('kdm,km->kd', C0, e)).T
    Csc = (C0 * np.sqrt(0.5)).transpose(1, 0, 2).reshape(d, Kc * l)  # k-major

    # merged [H | Csc | G] per d-row: one constant-table DMA covers all
    # matmul weights; partition-contiguous [128, c, 768] layout
    wallg = np.concatenate([H, Csc, G], axis=1).astype(ml_dtypes.float8_e4m3)
    wallg_pc = np.ascontiguousarray(
        wallg.reshape(4, 128, WG_COLS).transpose(1, 0, 2))        # (128,4,768)
    cfill = np.tile(CONST.astype(np.float16)[None, :], (128, 1))  # (128, K)
    return wallg_pc.reshape(128, 4 * WG_COLS), cfill


def _tile_xt_swi(xt, dtype):
    """DoubleRowSwInterleave stationary layout: per (tile i, chunk-pair q),
    a [128, 256] block E with E[p, 2j+c] = xt[(2q+c)*128+p, i*128 + (127-j)]
    (pairs interleaved, columns reversed).  Returns [128, n_sub, 512]."""
    d, n = xt.shape
    n_sub = n // 128
    a = xt.reshape(2, 2, 128, n_sub, 128)       # [q, c, p, i, nn]
    a = a[:, :, :, :, ::-1]                     # reverse sample cols -> j
    a = a.transpose(2, 3, 0, 4, 1)              # [p, i, q, j, c]
    return np.ascontiguousarray(a.astype(dtype)).reshape(128, n_sub, 512)


def build_nc(n_per_core=N_PER_CORE):
    """Build and compile the Bass module for one core (SPMD across 8)."""
    import concourse.bacc as bacc
    import concourse.tile as tile
    import concourse.mybir as mybir

    f32 = mybir.dt.float32
    f16 = mybir.dt.float16
    f8 = mybir.dt.float8e4
    SWI = mybir.MatmulPerfMode.DoubleRowSwInterleave
    n_sub = n_per_core // 128
    assert n_per_core % 128 == 0

    nc = bacc.Bacc("TRN2", target_bir_lowering=False, debug=False,
                   enable_asserts=False, num_devices=N_CORES)
    # xz: per tile i, [v=2 (x | x^2), 512] interleaved SWI blocks
    xz_dram = nc.dram_tensor("xz", (128, n_sub * 2 * 512), f8,
                             kind="ExternalInput")
    wallg_dram = nc.dram_tensor("wallg", (128, 4 * WG_COLS), f8,
                                kind="ExternalInput")
    c_dram = nc.dram_tensor("cfill", (128, K), f16, kind="ExternalInput")
    # partition-major output: out[p, i*K+k]; host untransposes
    out_dram = nc.dram_tensor("out", (128, n_sub * K), f16,
                              kind="ExternalOutput")

    SB = 8                       # tiles per superblock (batched DMA)
    n_super = n_sub // SB
    assert n_sub % SB == 0
    xz_v = xz_dram.ap().rearrange("p (s j v c n) -> p s j v c n",
                                  j=SB, v=2, c=4, n=128)
    out_v = out_dram.ap().rearrange("p (s j k) -> p s j k", j=SB, k=K)

    wca = K + NGA * L_FAC  # 384

    with tile.TileContext(nc) as tc, nc.allow_low_precision("fp16 within rel tolerance"):
        with (
            tc.tile_pool(name="wpool", bufs=1) as wpool,
            tc.tile_pool(name="xpool", bufs=3) as xpool,
            tc.tile_pool(name="opool", bufs=2) as opool,
            tc.tile_pool(name="ppool", bufs=4, space="PSUM") as ppool,
        ):
            wallg_sb = wpool.tile([128, 4, WG_COLS], f8)
            c_sb = wpool.tile([128, K], f16)

            # sbuf tiles for superblock 0 (filled per-tile for low latency)
            xz0 = xpool.tile([128, SB, 2, 4, 128], f8, tag="xz")
            xz1 = xpool.tile([128, SB, 2, 4, 128], f8, tag="xz")

            # scalar HWDGE queue (head only -- a waiting DMA at this FIFO's
            # head would block the ACT squares): wall+G halves, then sb1.
            # All have zero-wait deps.  DMA queues are descriptor-limited
            # (~10.5ns/packet, packet = per-partition contiguous run), so
            # runs must be >=2KB to sustain rate.
            wg_flat = wallg_sb[:].rearrange("p c m -> p (c m)")
            nc.scalar.dma_start(out=wg_flat[:, 0:2 * WG_COLS],
                                in_=wallg_dram.ap()[:, 0:2 * WG_COLS])
            nc.scalar.dma_start(out=wg_flat[:, 2 * WG_COLS:4 * WG_COLS],
                                in_=wallg_dram.ap()[:, 2 * WG_COLS:4 * WG_COLS])
            # sync HWDGE queue: tile-pair fills for superblock 0, paced by
            # sync-engine NOPs so their transfers SERIALIZE (concurrent HWDGE
            # sub-queues otherwise round-robin all pending DMAs, spreading
            # every completion to the end of the head).  cfill rides between
            # fills (needed by tile0's STT ~1us after T0).
            nc.sync.dma_start(out=xz0[:, 0:2], in_=xz_v[:, 0, 0:2])
            nc.sync.dma_start(out=xz0[:, 2:4], in_=xz_v[:, 0, 2:4])
            nc.sync.dma_start(out=c_sb[:], in_=c_dram.ap())
            nc.sync.dma_start(out=xz0[:, 4:6], in_=xz_v[:, 0, 4:6])
            nc.sync.dma_start(out=xz0[:, 6:SB], in_=xz_v[:, 0, 6:SB])

            # sb1 fills on the scalar queue, gated behind tile-01's data via
            # tiny gpsimd copies (8-byte overlap inside each dst, overwritten
            # by the real DMA) so sb1's transfers don't steal head bandwidth
            # from the critical wallg+sb0 fetches.
            nc.gpsimd.tensor_copy(xz1[:, 4, 0, 0, 0:8], xz0[:, 0, 0, 0, 0:8])
            nc.scalar.dma_start(out=xz1[:, 0:4], in_=xz_v[:, 1, 0:4])
            nc.scalar.dma_start(out=xz1[:, 4:SB], in_=xz_v[:, 1, 4:SB])

            # HAM warmup: keep the PE busy with zero-matmuls until the first
            # real matmul's data lands, so HAM reaches 8/8 with no idle gap
            zwarm = wpool.tile([128, 128], f8)
            nc.any.memset(zwarm[:], 0.0)
            pwarm = ppool.tile([128, 1024], f32, tag="ps")
            for w in range(31):
                nc.tensor.matmul(pwarm[:, 0:128], zwarm[:], zwarm[:],
                                 start=True, stop=True)

            # shared square tile (squares only), manually multi-buffered
            NSQ = 6
            sq = wpool.tile([128, NSQ, K, L_FAC], f16)
            # r1: [/, b, k, 0:5] pair-sums | 5: H+G+CONST
            NR1 = 6
            r1 = wpool.tile([128, NR1, K, 6], f16)

            for s in range(n_super):
                if s == 0:
                    xz_sb = xz0
                elif s == 1:
                    xz_sb = xz1
                else:
                    xz_sb = xpool.tile([128, SB, 2, 4, 128], f8, tag="xz")
                    nc.sync.dma_start(out=xz_sb[:], in_=xz_v[:, s])
                out_sb = opool.tile([128, SB, K], f16, tag="out")

                for j in range(SB):
                    i = s * SB + j
                    # one 2-bank psum tile: [H+G 64 | Csc_a 320 | pad |
                    # Csc_b 320 at +576]; bank0 = a-group, bank1 = b-group
                    psum = ppool.tile([128, 1024], f32, tag="ps")

                    def mm_a2(q, start, stop):
                        nc.tensor.matmul(psum[:, 0:wca],
                                         xz_sb[:, j, 0, 2*q:2*q+2, :],
                                         wallg_sb[:, 2*q:2*q+2, 0:wca],
                                         start=start, stop=stop,
                                         perf_mode=SWI)

                    def mm_b2(q, start, stop):
                        nc.tensor.matmul(psum[:, 576:896],
                                         xz_sb[:, j, 0, 2*q:2*q+2, :],
                                         wallg_sb[:, 2*q:2*q+2, wca:WALL_COLS],
                                         start=start, stop=stop,
                                         perf_mode=SWI,
                                         skip_group_check=True)

                    def mm_g2(q):
                        nc.tensor.matmul(psum[:, 0:K],
                                         xz_sb[:, j, 1, 2*q:2*q+2, :],
                                         wallg_sb[:, 2*q:2*q+2, WALL_COLS:WG_COLS],
                                         start=False, stop=False,
                                         perf_mode=SWI,
                                         skip_group_check=True)

                    mm_a2(0, True, False)
                    mm_b2(0, True, False)
                    mm_g2(0)
                    mm_g2(1)
                    mm_a2(1, False, True)
                    mm_b2(1, False, True)

                    # evacuate H+G (+CONST) into r1 col 5 early so psum
                    # frees after the squares (PE would stall on psum bufs)
                    r1_i = r1[:, i % NR1]
                    nc.vector.scalar_tensor_tensor(
                        r1_i[:, :, 5], psum[:, 0:K], 1.0, c_sb[:],
                        mybir.AluOpType.mult, mybir.AluOpType.add)

                    # all 640 squares in ONE activation: psum cols
                    # [64:384] and [576:896] = two 320-blocks, 512 apart
                    sq_i = sq[:, i % NSQ]
                    psq = (psum[:]
                           .rearrange("p (b x) -> p b x", b=2)[:, :, 64:384]
                           .rearrange("p b (g t) -> p b g t", t=L_FAC))
                    nc.scalar.square(
                        sq_i[:, :, 0:L_FAC].rearrange(
                            "p (b g) t -> p b g t", b=2), psq)

                    nc.gpsimd.tensor_add(r1_i[:, :, 0:5], sq_i[:, :, 0:5],
                                         sq_i[:, :, 5:10])

                    # single reduce folds pair-sums + CONST + H+G -> output
                    nc.vector.reduce_sum(out_sb[:, j], r1_i[:],
                                         axis=mybir.AxisListType.X)

                if s == n_super - 1:
                    # split the final out-DMA so the first half ships while
                    # the last tiles' post-chain drains (shorter tail)
                    h = SB // 2
                    nc.sync.dma_start(out=out_v[:, s, 0:h], in_=out_sb[:, 0:h])
                    nc.sync.dma_start(out=out_v[:, s, h:SB], in_=out_sb[:, h:SB])
                else:
                    nc.sync.dma_start(out=out_v[:, s], in_=out_sb[:])

    nc.compile()
    return nc


_NC_CACHE = {}


def _get_nc(n_per_core=N_PER_CORE):
    if n_per_core not in _NC_CACHE:
        _NC_CACHE[n_per_core] = build_nc(n_per_core)
    return _NC_CACHE[n_per_core]


def _install_ntff_hook():
    """Provide the antenv.axon_hooks shim so trace=True can capture NTFFs."""
    import sys
    if "antenv.axon_hooks" in sys.modules:
        return
    import types
    import ctypes
    import contextlib

    so_path = "/opt/axon/libaxon_pjrt.so"
    lib = ctypes.CDLL(so_path)
    if not hasattr(lib, "axon_start_nrt_profile"):
        return
    lib.axon_start_nrt_profile.argtypes = [ctypes.POINTER(ctypes.c_int64), ctypes.c_size_t]
    lib.axon_start_nrt_profile.restype = ctypes.c_int64
    lib.axon_stop_nrt_profile.argtypes = [ctypes.c_char_p]
    lib.axon_stop_nrt_profile.restype = ctypes.c_int64

    @contextlib.contextmanager
    def _hook(output_dir, device_ids):
        import jax
        jax.devices()
        if device_ids:
            ids = (ctypes.c_int64 * len(device_ids))(*device_ids)
            rc = lib.axon_start_nrt_profile(ids, len(device_ids))
        else:
            rc = lib.axon_start_nrt_profile(None, 0)
        if rc != 0:
            raise RuntimeError(f"axon_start_nrt_profile rc={rc}")
        try:
            yield
        finally:
            n = lib.axon_stop_nrt_profile(str(output_dir).encode())
            print(f"ntff profile: {n} file(s) written to {output_dir}")

    mod = types.ModuleType("antenv.axon_hooks")
    mod.get_axon_ntff_profile_hook = lambda: _hook
    mod.set_axon_ntff_profile_hook = lambda h: None
    sys.modules["antenv.axon_hooks"] = mod


def kernel(x, MU, A, D, PI, trace=False):
    from concourse.bass_utils import run_bass_kernel_spmd
    if trace:
        try:
            _install_ntff_hook()
        except Exception as e:
            print(f"ntff hook install failed: {e}")
            trace = False

    x = np.asarray(x)
    wallg, cfill = host_prep(MU, A, D, PI)
    nc = _get_nc()

    n_sub = N_PER_CORE // 128
    in_maps = []
    for c in range(N_CORES):
        xs = np.ascontiguousarray(x[c * N_PER_CORE:(c + 1) * N_PER_CORE, :].T)
        xs = xs.astype(np.float32)
        xt = _tile_xt_swi(xs, ml_dtypes.float8_e4m3)        # [128, n_sub, 512]
        x2t = _tile_xt_swi(xs * xs, ml_dtypes.float8_e4m3)  # [128, n_sub, 512]
        xz = np.stack([xt, x2t], axis=2)                    # [128, n_sub, 2, 512]
        in_maps.append({
            "xz": np.ascontiguousarray(xz).reshape(128, n_sub * 2 * 512),
            "wallg": wallg, "cfill": cfill,
        })

    res = run_bass_kernel_spmd(nc, in_maps, list(range(N_CORES)), trace=trace)
    outs = []
    for c in range(N_CORES):
        o = res.results[c]["out"].reshape(128, n_sub, K)
        outs.append(o.transpose(1, 0, 2).reshape(N_PER_CORE, K).astype(np.float32))
    out = np.concatenate(outs, axis=0)
    if trace:
        kernel.last_exec_time_ns = res.exec_time_ns
        kernel.last_results = res
    return out
